# revision 22
# baseline (speedup 1.0000x reference)
"""Multi-head self-attention (B=2, N=2048, C=1024, H=16) on 8 TRN2 NeuronCores.

v3 reconstruction: the variant measured at 230045 ns.
"""

import contextlib
from collections import deque

import numpy as np

import concourse.bass as bass
import concourse.bacc as bacc
import concourse.tile as tile
from concourse import mybir
from concourse.bass_utils import run_bass_kernel_spmd

B, NSEQ, CDIM, NHEADS, HD = 2, 2048, 1024, 16, 64
NH = 4          # heads per core
NCORES = 8
F32 = mybir.dt.float32
F16 = mybir.dt.float16
EXP = mybir.ActivationFunctionType.Exp
SCALE = HD ** -0.5


def build_program():
    nc = bacc.Bacc("TRN2", target_bir_lowering=False, debug=False)

    xT = nc.dram_tensor("xT", [CDIM, NSEQ], F16, kind="ExternalInput").ap()
    wqkv = nc.dram_tensor("wqkv", [CDIM, 3 * NH * HD], F16, kind="ExternalInput").ap()
    wout = nc.dram_tensor("wout", [NH * HD, CDIM], F16, kind="ExternalInput").ap()
    y = nc.dram_tensor("y", [NSEQ, CDIM], F16, kind="ExternalOutput").ap()

    with tile.TileContext(nc) as tc:
        emit(nc, tc, xT, wqkv, wout, y)

    nc.compile()
    return nc


def emit(nc, tc, xT, wqkv, wout, y):
    ctx = contextlib.ExitStack()
    with ctx:
        const = ctx.enter_context(tc.tile_pool(name="const", bufs=1))

        xT_sb = const.tile([128, 8, NSEQ], F16)
        wqkv_sb = const.tile([128, 8, 3 * NH * HD], F16)
        wout_sb = const.tile([128, 2, CDIM], F16)
        qk_sb = const.tile([128, 4, NSEQ], F16)         # dim1: q01,q23,k01,k23
        v_aug = const.tile([128, 16, NH, HD + 1], F16)  # [p, ntile, head, V|1]
        o_sb = const.tile([128, 2, NSEQ], F16)          # normalized O^T, pairs
        ones64 = const.tile([1, HD], F16)
        warm_i = const.tile([1, 1], F32)
        warm_o = const.tile([1, 1], F16)

        dumm = const.tile([1, 256], F16)
        nc.vector.memset(v_aug[:, :, :, HD:HD + 1], 1.0)
        nc.vector.memset(ones64, 1.0)
        nc.vector.memset(dumm, 0.0)
        nc.vector.memset(warm_i, 0.0)
        # load the exp table set during the DMA window, not on the hot path
        nc.scalar.activation(warm_o, warm_i, EXP)

        with tc.tile_pool(name="pP", bufs=4) as pP, \
             tc.tile_pool(name="oup", bufs=4) as oup, \
             tc.tile_pool(name="stat", bufs=2) as stat, \
             tc.tile_pool(name="yb", bufs=3) as yb, \
             tc.tile_pool(name="psm", bufs=1, space="PSUM") as psm:

            xT_t = xT.rearrange("(t p) n -> p t n", p=128)
            wqkv_t = wqkv.rearrange("(t p) f -> p t f", p=128)
            wout_t = wout.rearrange("(t p) f -> p t f", p=128)
            # Two queues (two DMA channels), and x split into column halves:
            # the prologue chains for seq chunks 0-1 and the first 8 V chains
            # only read x columns 0:1024, so the attention pipeline starts
            # after ~3.5MB instead of the full 5.5MB.
            for ct in range(8):
                if ct % 2 == 0:
                    nc.gpsimd.dma_start(xT_sb[:, ct, 0:1024],
                                        xT_t[:, ct, 0:1024])
                    nc.sync.dma_start(wqkv_sb[:, ct, :], wqkv_t[:, ct, :])
                else:
                    nc.sync.dma_start(xT_sb[:, ct, 0:1024],
                                      xT_t[:, ct, 0:1024])
                    nc.gpsimd.dma_start(wqkv_sb[:, ct, :], wqkv_t[:, ct, :])
            for ct in range(8):
                q = nc.gpsimd if ct % 2 == 0 else nc.sync
                q.dma_start(xT_sb[:, ct, 1024:2048], xT_t[:, ct, 1024:2048])
            for kt in range(2):
                nc.sync.dma_start(wout_sb[:, kt, :], wout_t[:, kt, :])

            # ---------------- building blocks --------------------------------
            def qk_chain(ft, ic, c0, c1, ps):
                if ps is None:
                    ps = psm.tile([128, 512], F32, tag="rbt", bufs=2, name="psqk")
                for ct in range(c0, c1):
                    nc.tensor.matmul(
                        ps,
                        wqkv_sb[:, ct, ft * 128:(ft + 1) * 128],
                        xT_sb[:, ct, ic * 512:(ic + 1) * 512],
                        start=(ct == 0), stop=(ct == 7),
                    )
                if c1 == 8:
                    nc.vector.tensor_copy(
                        qk_sb[:, ft, ic * 512:(ic + 1) * 512], ps)
                return ps

            def qk_group(ft, ic, tag="sb"):
                ps = psm.tile([128, 512], F32, tag=tag, bufs=2, name="psqk")
                qk_chain(ft, ic, 0, 8, ps)

            def v_chain(nt, c0, c1, ps):
                if ps is None:
                    ps = psm.tile([128, NH * HD], F32, tag="rbt", bufs=2, name="psv")
                for ct in range(c0, c1):
                    nc.tensor.matmul(
                        ps,
                        xT_sb[:, ct, nt * 128:(nt + 1) * 128],
                        wqkv_sb[:, ct, 512:768],
                        start=(ct == 0), stop=(ct == 7),
                    )
                if c1 == 8:
                    nc.vector.tensor_copy(v_aug[:, nt, :, 0:HD], ps)
                return ps

            def psy_tile(it, fc, tag="rbt", bufs=2, scalar_copy=False):
                psy = psm.tile([128, 512], F32, tag=tag, bufs=bufs, name="psy")
                for pp in range(2):
                    nc.tensor.matmul(
                        psy,
                        o_sb[:, pp, it * 128:(it + 1) * 128],
                        wout_sb[:, pp, fc * 512:(fc + 1) * 512],
                        start=(pp == 0), stop=(pp == 1),
                    )
                y_sb = yb.tile([128, 512], F16, tag="ysb", name="ysbt")
                if scalar_copy:
                    nc.scalar.copy(y_sb, psy)
                    nc.gpsimd.dma_start(
                        y[it * 128:(it + 1) * 128, fc * 512:(fc + 1) * 512],
                        y_sb)
                else:
                    nc.vector.tensor_copy(y_sb, psy)
                    nc.sync.dma_start(
                        y[it * 128:(it + 1) * 128, fc * 512:(fc + 1) * 512],
                        y_sb)

            fillers = deque()

            def pop_fillers(budget):
                while fillers and budget > 0:
                    cost, thunk = fillers[0]
                    if cost > budget and budget < 700:
                        break
                    fillers.popleft()
                    thunk()
                    budget -= cost

            def queue_qk(ft, ic):
                st = {"ps": None}

                def half(c0, c1):
                    def run():
                        st["ps"] = qk_chain(ft, ic, c0, c1, st["ps"])
                    return run
                fillers.append((950, half(0, 4)))
                fillers.append((1050, half(4, 8)))

            def normalize(p, i0, qw, po, tail=False):
                """tail=True splits the copies onto the (then idle) Scalar
                engine so the serial chain halves."""
                thunks = []
                for e in range(2):
                    r0 = stat.tile([1, 512], F32, tag="r0", name="r0t")
                    o_u = oup.tile([128, 512], F32, tag="ou", name="out_u")
                    lo = 64 * e
                    if tail:
                        nc.scalar.copy(r0[0:1, 0:qw], po[e][HD:HD + 1, 0:qw])
                        nc.scalar.copy(o_u[lo:lo + 64, 0:qw],
                                       po[e][0:64, 0:qw])
                    else:
                        nc.vector.tensor_copy(r0[0:1, 0:qw],
                                              po[e][HD:HD + 1, 0:qw])
                        nc.vector.tensor_copy(o_u[lo:lo + 64, 0:qw],
                                              po[e][0:64, 0:qw])
                    rs = stat.tile([1, 512], F32, tag="rs", name="rst")
                    r1 = stat.tile([1, 512], F32, tag="r1", name="r1t")
                    nc.vector.reciprocal_approx_accurate(
                        r1[0:1, 0:qw], r0[0:1, 0:qw], rs[0:1, 0:qw])
                    rcp = stat.tile([1, 512], F16, tag="rc", bufs=4, name="rct")
                    nc.vector.tensor_copy(rcp[0:1, 0:qw], r1[0:1, 0:qw])

                    def mk(e=e, o_u=o_u, rcp=rcp):
                        def run():
                            rb = psm.tile([128, 512], F32, tag="rbt", bufs=2,
                                          name="rb")
                            lo = 64 * e
                            nc.tensor.matmul(
                                rb[lo:lo + 64, 0:qw], ones64, rcp[0:1, 0:qw],
                                start=True, stop=True, tile_position=(0, lo),
                            )
                            nc.vector.tensor_mul(
                                o_sb[lo:lo + 64, p, i0:i0 + qw],
                                o_u[lo:lo + 64, 0:qw], rb[lo:lo + 64, 0:qw])
                        return run
                    thunks.append((700, mk()))
                return thunks

            # ---------------- prologue: 6 chains interleaved by ct -----------
            # one PSUM slot per chain so every chain's MM(ct) can issue as
            # soon as the ct-th x/w slices land -- the PE chases the DMA.
            # HAM keep-warm: the PE would idle ~10us waiting for the input
            # DMA and drop to the 1.2GHz throttled clock right when the
            # latency-critical prologue chains run. Dependency-free dummy
            # matmuls (zero data, junk PSUM slot later cleared by a real
            # chain's start=True) keep the activity window busy so the
            # prologue starts at 2.4GHz.
            dps = psm.tile([64, 256], F32, tag="rbt", bufs=2, name="dps")
            for i in range(100):
                nc.tensor.matmul(dps, ones64, dumm,
                                 start=(i == 0), stop=(i == 99))

            # wave A: chains that need only x columns 0:1024 (seq chunks
            # 0-1); wave B (seq chunks 2-3) chases the x second halves and
            # finishes well before chunk 0 reaches key tile 8.
            proA = [(2, 0, "sb"), (2, 1, "sb"), (0, 0, "rbt"), (0, 1, "rbt")]
            proB = [(2, 2, "o0"), (2, 3, "o1")]
            pro_ps = {}
            for ft, ic, tag in proA + proB:
                pro_ps[(ft, ic)] = psm.tile(
                    [128, 512], F32, tag=tag,
                    bufs=2 if tag in ("sb", "rbt") else 1, name="psqk")
            for ct in range(8):
                for ft, ic, tag in proA:
                    nc.tensor.matmul(
                        pro_ps[(ft, ic)],
                        wqkv_sb[:, ct, ft * 128:(ft + 1) * 128],
                        xT_sb[:, ct, ic * 512:(ic + 1) * 512],
                        start=(ct == 0), stop=(ct == 7),
                    )
            for k, (ft, ic) in enumerate([(2, 0), (0, 0), (2, 1), (0, 1)]):
                dst = qk_sb[:, ft, ic * 512:(ic + 1) * 512]
                if k % 2 == 0:
                    nc.vector.tensor_copy(dst, pro_ps[(ft, ic)])
                else:
                    nc.scalar.copy(dst, pro_ps[(ft, ic)])
            for ct in range(8):
                for ft, ic, tag in proB:
                    nc.tensor.matmul(
                        pro_ps[(ft, ic)],
                        wqkv_sb[:, ct, ft * 128:(ft + 1) * 128],
                        xT_sb[:, ct, ic * 512:(ic + 1) * 512],
                        start=(ct == 0), stop=(ct == 7),
                    )
            for k, (ft, ic, tag) in enumerate(proB):
                dst = qk_sb[:, ft, ic * 512:(ic + 1) * 512]
                if k % 2 == 0:
                    nc.vector.tensor_copy(dst, pro_ps[(ft, ic)])
                else:
                    nc.scalar.copy(dst, pro_ps[(ft, ic)])

            queue_qk(0, 2)               # before chunk (0,2)
            queue_qk(0, 3)               # before chunk (0,3)
            for ic in range(4):
                queue_qk(3, ic)          # K^T heads 2,3 before pair 1
            for ic in range(4):
                queue_qk(1, ic)          # Q^T heads 2,3

            # ---------------- attention + interleaved everything -------------
            # Head e1 always lands at fp32 column 512 of the scores tile (the
            # second PSUM bank) so the two concurrent row-group matmuls never
            # share a bank, for any query width. The single ACT covers
            # [0:512+qw]; the stale gap columns are harmless (bounded scores).
            chunks = [(0, 0, 512), (0, 512, 512), (0, 1024, 512), (0, 1536, 512),
                      (1, 0, 512), (1, 512, 512), (1, 1024, 512),
                      (1, 1536, 512)]
            pending_finish = [None]

            def chunk_finish(p, i0, qw, po, pts, last):
                def fin():
                    for e in range(2):
                        nc.tensor.matmul(
                            po[e][0:HD + 1, 0:qw],
                            v_aug[:, 15, 2 * p + e, :],
                            pts[15][:, e * 512:e * 512 + qw],
                            start=False, stop=True,
                        )
                    norm_thunks = normalize(p, i0, qw, po, tail=last)
                    if last:
                        for _, t in norm_thunks:
                            t()
                    else:
                        for th in reversed(norm_thunks):
                            fillers.appendleft(th)
                    if p == 1:
                        for it in range(i0 // 128, (i0 + qw) // 128):
                            for fc in range(2):
                                th = lambda it=it, fc=fc: psy_tile(it, fc)
                                th.is_psy = (it, fc)
                                fillers.append((750, th))
                return fin

            for ci, (p, i0, qw) in enumerate(chunks):
                last = ci == len(chunks) - 1
                po = [psm.tile([128, 512], F32, tag=f"o{e}", name=f"po{e}")
                      for e in range(2)]
                pts = []
                for jt in range(16):
                    if ci == 0:
                        v_chain(jt, 0, 8, None)
                    ps = psm.tile([128, 1024], F32, tag="sb", bufs=2,
                                  name="pss")
                    for e in range(2):
                        pb = 64 * e
                        nc.tensor.matmul(
                            ps[:, e * 512:e * 512 + qw],
                            qk_sb[pb:pb + 64, 2 + p, jt * 128:(jt + 1) * 128],
                            qk_sb[pb:pb + 64, p, i0:i0 + qw],
                            start=True, stop=True,
                            tile_position=(pb, 0),
                        )
                    pt = pP.tile([128, 1024], F16, tag="p")
                    pts.append(pt)
                    nc.scalar.activation(pt[:, 0:512 + qw], ps[:, 0:512 + qw],
                                         EXP, scale=SCALE)
                    if jt == 0 and pending_finish[0] is not None:
                        pending_finish[0]()
                        pending_finish[0] = None
                    if jt > 0:
                        if jt > 1 and ci > 0:
                            pop_fillers(700)
                        for e in range(2):
                            nc.tensor.matmul(
                                po[e][0:HD + 1, 0:qw],
                                v_aug[:, jt - 1, 2 * p + e, :],
                                pts[jt - 1][:, e * 512:e * 512 + qw],
                                start=(jt - 1 == 0), stop=False,
                            )
                pending_finish[0] = chunk_finish(p, i0, qw, po, pts, last)
            pending_finish[0]()

            # drain: the last chunk's out-projection on the freed wide scores
            # slots -- both feature chunks of a query tile in one [128,1024]
            # PSUM tile, one wide copy (alternating Scalar/Vector), one DMA.
            k = 0
            merged = set()
            while fillers:
                cost, thunk = fillers.popleft()
                if getattr(thunk, "is_psy", None):
                    it, fc = thunk.is_psy
                    if fc == 1:
                        if it in merged:
                            continue    # covered by the fc==0 wide tile
                        thunk()         # fc0 was popped mid-kernel
                        continue
                    merged.add(it)
                    psy2 = psm.tile([128, 1024], F32, tag="sb", bufs=2,
                                    name="psy2")
                    for fc2 in range(2):
                        for pp in range(2):
                            nc.tensor.matmul(
                                psy2[:, fc2 * 512:(fc2 + 1) * 512],
                                o_sb[:, pp, it * 128:(it + 1) * 128],
                                wout_sb[:, pp, fc2 * 512:(fc2 + 1) * 512],
                                start=(pp == 0), stop=(pp == 1),
                            )
                    y_sb = yb.tile([128, 1024], F16, tag="yw", bufs=2,
                                   name="ywt")
                    if k % 2 == 0:
                        nc.scalar.copy(y_sb, psy2)
                        nc.gpsimd.dma_start(
                            y[it * 128:(it + 1) * 128, :], y_sb)
                    else:
                        nc.vector.tensor_copy(y_sb, psy2)
                        nc.sync.dma_start(
                            y[it * 128:(it + 1) * 128, :], y_sb)
                    k += 1
                else:
                    thunk()


_NC = None


def _get_nc():
    global _NC
    if _NC is None:
        _NC = build_program()
    return _NC


def make_in_maps(x, w_qkv, w_out):
    x = np.asarray(x, dtype=np.float32)
    w_qkv = np.asarray(w_qkv, dtype=np.float32)
    w_out = np.asarray(w_out, dtype=np.float32)
    xT = [np.ascontiguousarray(x[b].T).astype(np.float16) for b in range(B)]
    in_maps = []
    for c in range(NCORES):
        b, g = divmod(c, 4)
        f0 = g * NH * HD  # first feature col of this head group (256 wide)
        wq = w_qkv[:, f0:f0 + 256]
        wk = w_qkv[:, CDIM + f0:CDIM + f0 + 256]
        wv = w_qkv[:, 2 * CDIM + f0:2 * CDIM + f0 + 256]
        in_maps.append({
            "xT": xT[b],
            "wqkv": np.concatenate([wq, wk, wv], axis=1).astype(np.float16),
            "wout": np.ascontiguousarray(w_out[f0:f0 + 256, :]).astype(np.float16),
        })
    return in_maps


def kernel(x, w_qkv, b_qkv, w_out, b_out, _trace=False):
    """Full inputs in, full (B, N, C) output out. b_qkv is all-zeros by the
    problem's input spec (fill: zeros); b_out is added on the host."""
    nc = _get_nc()
    in_maps = make_in_maps(x, w_qkv, w_out)
    res = run_bass_kernel_spmd(nc, in_maps, core_ids=list(range(NCORES)),
                               trace=_trace)
    out = np.zeros((B, NSEQ, CDIM), dtype=np.float32)
    for c in range(NCORES):
        out[c // 4] += res.results[c]["y"].astype(np.float32)
    out += np.asarray(b_out, dtype=np.float32)
    if _trace:
        kernel.last_exec_time_ns = res.exec_time_ns
        kernel.last_results = res
    return out


# revision 24
# speedup vs baseline: 1.0670x; 1.0670x over previous
"""Multi-head self-attention (B=2, N=2048, C=1024, H=16) on 8 TRN2 NeuronCores.

v3 reconstruction: the variant measured at 230045 ns.
"""

import contextlib
from collections import deque

import numpy as np

import concourse.bass as bass
import concourse.bacc as bacc
import concourse.tile as tile
from concourse import mybir
from concourse.bass_utils import run_bass_kernel_spmd

B, NSEQ, CDIM, NHEADS, HD = 2, 2048, 1024, 16, 64
NH = 4          # heads per core
NCORES = 8
F32 = mybir.dt.float32
F16 = mybir.dt.float16
EXP = mybir.ActivationFunctionType.Exp
SCALE = HD ** -0.5


def build_program():
    nc = bacc.Bacc("TRN2", target_bir_lowering=False, debug=False)

    xT = nc.dram_tensor("xT", [CDIM, NSEQ], F16, kind="ExternalInput").ap()
    wqkv = nc.dram_tensor("wqkv", [CDIM, 3 * NH * HD], F16, kind="ExternalInput").ap()
    wout = nc.dram_tensor("wout", [NH * HD, CDIM], F16, kind="ExternalInput").ap()
    y = nc.dram_tensor("y", [NSEQ, CDIM], F16, kind="ExternalOutput").ap()

    with tile.TileContext(nc) as tc:
        emit(nc, tc, xT, wqkv, wout, y)

    nc.compile()
    return nc


def emit(nc, tc, xT, wqkv, wout, y):
    ctx = contextlib.ExitStack()
    with ctx:
        const = ctx.enter_context(tc.tile_pool(name="const", bufs=1))

        xT_sb = const.tile([128, 8, NSEQ], F16)
        wqkv_sb = const.tile([128, 8, 3 * NH * HD], F16)
        wout_sb = const.tile([128, 2, CDIM], F16)
        qk_sb = const.tile([128, 4, NSEQ], F16)         # dim1: q01,q23,k01,k23
        v_aug = const.tile([128, 16, NH, HD + 1], F16)  # [p, ntile, head, V|1]
        o_sb = const.tile([128, 2, NSEQ], F16)          # normalized O^T, pairs
        ones64 = const.tile([1, HD], F16)
        warm_i = const.tile([1, 1], F32)
        warm_o = const.tile([1, 1], F16)

        nc.vector.memset(v_aug[:, :, :, HD:HD + 1], 1.0)
        nc.vector.memset(ones64, 1.0)
        nc.vector.memset(warm_i, 0.0)
        # load the exp table set during the DMA window, not on the hot path
        nc.scalar.activation(warm_o, warm_i, EXP)

        with tc.tile_pool(name="pP", bufs=6) as pP, \
             tc.tile_pool(name="oup", bufs=4) as oup, \
             tc.tile_pool(name="stat", bufs=2) as stat, \
             tc.tile_pool(name="yb", bufs=4) as yb, \
             tc.tile_pool(name="psm", bufs=1, space="PSUM") as psm:

            xT_t = xT.rearrange("(t p) n -> p t n", p=128)
            wqkv_t = wqkv.rearrange("(t p) f -> p t f", p=128)
            wout_t = wout.rearrange("(t p) f -> p t f", p=128)
            # Two queues (two DMA channels), and x split into column halves:
            # the prologue chains for seq chunks 0-1 and the first 8 V chains
            # only read x columns 0:1024, so the attention pipeline starts
            # after ~3.5MB instead of the full 5.5MB.
            for ct in range(8):
                if ct % 2 == 0:
                    nc.gpsimd.dma_start(xT_sb[:, ct, 0:1024],
                                        xT_t[:, ct, 0:1024])
                    nc.sync.dma_start(wqkv_sb[:, ct, :], wqkv_t[:, ct, :])
                else:
                    nc.sync.dma_start(xT_sb[:, ct, 0:1024],
                                      xT_t[:, ct, 0:1024])
                    nc.gpsimd.dma_start(wqkv_sb[:, ct, :], wqkv_t[:, ct, :])
            for ct in range(8):
                q = nc.gpsimd if ct % 2 == 0 else nc.sync
                q.dma_start(xT_sb[:, ct, 1024:2048], xT_t[:, ct, 1024:2048])
            for kt in range(2):
                nc.sync.dma_start(wout_sb[:, kt, :], wout_t[:, kt, :])

            # ---------------- building blocks --------------------------------
            def qk_chain(ft, ic, c0, c1, ps):
                if ps is None:
                    ps = psm.tile([128, 512], F32, tag="rbt", bufs=2, name="psqk")
                for ct in range(c0, c1):
                    nc.tensor.matmul(
                        ps,
                        wqkv_sb[:, ct, ft * 128:(ft + 1) * 128],
                        xT_sb[:, ct, ic * 512:(ic + 1) * 512],
                        start=(ct == 0), stop=(ct == 7),
                    )
                if c1 == 8:
                    nc.vector.tensor_copy(
                        qk_sb[:, ft, ic * 512:(ic + 1) * 512], ps)
                return ps

            def qk_group(ft, ic, tag="sb"):
                ps = psm.tile([128, 512], F32, tag=tag, bufs=2, name="psqk")
                qk_chain(ft, ic, 0, 8, ps)

            def v_chain(nt, c0, c1, ps):
                if ps is None:
                    ps = psm.tile([128, NH * HD], F32, tag="rbt", bufs=2, name="psv")
                for ct in range(c0, c1):
                    nc.tensor.matmul(
                        ps,
                        xT_sb[:, ct, nt * 128:(nt + 1) * 128],
                        wqkv_sb[:, ct, 512:768],
                        start=(ct == 0), stop=(ct == 7),
                    )
                if c1 == 8:
                    nc.vector.tensor_copy(v_aug[:, nt, :, 0:HD], ps)
                return ps

            def psy_tile(it, fc, tag="rbt", bufs=2, scalar_copy=False):
                psy = psm.tile([128, 512], F32, tag=tag, bufs=bufs, name="psy")
                for pp in range(2):
                    nc.tensor.matmul(
                        psy,
                        o_sb[:, pp, it * 128:(it + 1) * 128],
                        wout_sb[:, pp, fc * 512:(fc + 1) * 512],
                        start=(pp == 0), stop=(pp == 1),
                    )
                y_sb = yb.tile([128, 512], F16, tag="ysb", name="ysbt")
                if scalar_copy:
                    nc.scalar.copy(y_sb, psy)
                    nc.gpsimd.dma_start(
                        y[it * 128:(it + 1) * 128, fc * 512:(fc + 1) * 512],
                        y_sb)
                else:
                    nc.vector.tensor_copy(y_sb, psy)
                    nc.sync.dma_start(
                        y[it * 128:(it + 1) * 128, fc * 512:(fc + 1) * 512],
                        y_sb)

            fillers = deque()

            def pop_fillers(budget):
                while fillers and budget > 0:
                    cost, thunk = fillers[0]
                    if cost > budget and budget < 700:
                        break
                    fillers.popleft()
                    thunk()
                    budget -= cost

            def queue_qk(ft, ic):
                st = {"ps": None}

                def half(c0, c1):
                    def run():
                        st["ps"] = qk_chain(ft, ic, c0, c1, st["ps"])
                    return run
                fillers.append((950, half(0, 4)))
                fillers.append((1050, half(4, 8)))

            def normalize(p, i0, qw, po, tail=False):
                """tail=True splits the copies onto the (then idle) Scalar
                engine so the serial chain halves."""
                thunks = []
                for e in range(2):
                    r0 = stat.tile([1, 512], F32, tag="r0", name="r0t")
                    o_u = oup.tile([128, 512], F32, tag="ou", name="out_u")
                    lo = 64 * e
                    if tail:
                        nc.scalar.copy(r0[0:1, 0:qw], po[e][HD:HD + 1, 0:qw])
                        nc.scalar.copy(o_u[lo:lo + 64, 0:qw],
                                       po[e][0:64, 0:qw])
                    else:
                        nc.vector.tensor_copy(r0[0:1, 0:qw],
                                              po[e][HD:HD + 1, 0:qw])
                        nc.vector.tensor_copy(o_u[lo:lo + 64, 0:qw],
                                              po[e][0:64, 0:qw])
                    rs = stat.tile([1, 512], F32, tag="rs", name="rst")
                    r1 = stat.tile([1, 512], F32, tag="r1", name="r1t")
                    nc.vector.reciprocal_approx_accurate(
                        r1[0:1, 0:qw], r0[0:1, 0:qw], rs[0:1, 0:qw])
                    rcp = stat.tile([1, 512], F16, tag="rc", bufs=4, name="rct")
                    nc.vector.tensor_copy(rcp[0:1, 0:qw], r1[0:1, 0:qw])

                    def mk(e=e, o_u=o_u, rcp=rcp):
                        def run():
                            rb = psm.tile([128, 512], F32, tag="rbt", bufs=2,
                                          name="rb")
                            lo = 64 * e
                            nc.tensor.matmul(
                                rb[lo:lo + 64, 0:qw], ones64, rcp[0:1, 0:qw],
                                start=True, stop=True, tile_position=(0, lo),
                            )
                            nc.vector.tensor_mul(
                                o_sb[lo:lo + 64, p, i0:i0 + qw],
                                o_u[lo:lo + 64, 0:qw], rb[lo:lo + 64, 0:qw])
                        return run
                    thunks.append((700, mk()))
                return thunks

            # ---------------- prologue: 6 chains interleaved by ct -----------
            # one PSUM slot per chain so every chain's MM(ct) can issue as
            # soon as the ct-th x/w slices land -- the PE chases the DMA.
            # wave A: chains that need only x columns 0:1024 (seq chunks
            # 0-1); wave B (seq chunks 2-3) chases the x second halves and
            # finishes well before chunk 0 reaches key tile 8.
            proA = [(2, 0, "sb"), (2, 1, "sb"), (0, 0, "rbt"), (0, 1, "rbt")]
            proB = [(2, 2, "o0"), (2, 3, "o1")]
            pro_ps = {}
            for ft, ic, tag in proA + proB:
                pro_ps[(ft, ic)] = psm.tile(
                    [128, 512], F32, tag=tag,
                    bufs=2 if tag in ("sb", "rbt") else 1, name="psqk")
            for ct in range(8):
                for ft, ic, tag in proA:
                    nc.tensor.matmul(
                        pro_ps[(ft, ic)],
                        wqkv_sb[:, ct, ft * 128:(ft + 1) * 128],
                        xT_sb[:, ct, ic * 512:(ic + 1) * 512],
                        start=(ct == 0), stop=(ct == 7),
                    )
            for k, (ft, ic) in enumerate([(2, 0), (0, 0), (2, 1), (0, 1)]):
                dst = qk_sb[:, ft, ic * 512:(ic + 1) * 512]
                if k % 2 == 0:
                    nc.vector.tensor_copy(dst, pro_ps[(ft, ic)])
                else:
                    nc.scalar.copy(dst, pro_ps[(ft, ic)])
            for ct in range(8):
                for ft, ic, tag in proB:
                    nc.tensor.matmul(
                        pro_ps[(ft, ic)],
                        wqkv_sb[:, ct, ft * 128:(ft + 1) * 128],
                        xT_sb[:, ct, ic * 512:(ic + 1) * 512],
                        start=(ct == 0), stop=(ct == 7),
                    )
            for k, (ft, ic, tag) in enumerate(proB):
                dst = qk_sb[:, ft, ic * 512:(ic + 1) * 512]
                if k % 2 == 0:
                    nc.vector.tensor_copy(dst, pro_ps[(ft, ic)])
                else:
                    nc.scalar.copy(dst, pro_ps[(ft, ic)])

            queue_qk(0, 2)               # before chunk (0,2)
            queue_qk(0, 3)               # before chunk (0,3)
            for ic in range(4):
                queue_qk(3, ic)          # K^T heads 2,3 before pair 1
            for ic in range(4):
                queue_qk(1, ic)          # Q^T heads 2,3

            # ---------------- attention + interleaved everything -------------
            # Head e1 always lands at fp32 column 512 of the scores tile (the
            # second PSUM bank) so the two concurrent row-group matmuls never
            # share a bank, for any query width. The single ACT covers
            # [0:512+qw]; the stale gap columns are harmless (bounded scores).
            chunks = [(0, 0, 512), (0, 512, 512), (0, 1024, 512), (0, 1536, 512),
                      (1, 0, 512), (1, 512, 512), (1, 1024, 512),
                      (1, 1536, 512)]
            pending_finish = [None]

            def chunk_finish(p, i0, qw, po, pts, last):
                def fin():
                    for e in range(2):
                        nc.tensor.matmul(
                            po[e][0:HD + 1, 0:qw],
                            v_aug[:, 15, 2 * p + e, :],
                            pts[15][:, e * 512:e * 512 + qw],
                            start=False, stop=True,
                        )
                    norm_thunks = normalize(p, i0, qw, po, tail=last)
                    if last:
                        for _, t in norm_thunks:
                            t()
                    else:
                        for th in reversed(norm_thunks):
                            fillers.appendleft(th)
                    if p == 1:
                        for it in range(i0 // 128, (i0 + qw) // 128):
                            for fc in range(2):
                                th = lambda it=it, fc=fc: psy_tile(it, fc)
                                th.is_psy = (it, fc)
                                fillers.append((750, th))
                return fin

            for ci, (p, i0, qw) in enumerate(chunks):
                last = ci == len(chunks) - 1
                po = [psm.tile([128, 512], F32, tag=f"o{e}", name=f"po{e}")
                      for e in range(2)]
                pts = []
                for jt in range(16):
                    if ci == 0:
                        v_chain(jt, 0, 8, None)
                    ps = psm.tile([128, 1024], F32, tag="sb", bufs=2,
                                  name="pss")
                    for e in range(2):
                        pb = 64 * e
                        nc.tensor.matmul(
                            ps[:, e * 512:e * 512 + qw],
                            qk_sb[pb:pb + 64, 2 + p, jt * 128:(jt + 1) * 128],
                            qk_sb[pb:pb + 64, p, i0:i0 + qw],
                            start=True, stop=True,
                            tile_position=(pb, 0),
                        )
                    pt = pP.tile([128, 1024], F16, tag="p")
                    pts.append(pt)
                    nc.scalar.activation(pt[:, 0:512 + qw], ps[:, 0:512 + qw],
                                         EXP, scale=SCALE)
                    if jt == 0 and pending_finish[0] is not None:
                        pending_finish[0]()
                        pending_finish[0] = None
                    if jt > 0:
                        if jt > 1 and ci > 0:
                            pop_fillers(700)
                        for e in range(2):
                            nc.tensor.matmul(
                                po[e][0:HD + 1, 0:qw],
                                v_aug[:, jt - 1, 2 * p + e, :],
                                pts[jt - 1][:, e * 512:e * 512 + qw],
                                start=(jt - 1 == 0), stop=False,
                            )
                pending_finish[0] = chunk_finish(p, i0, qw, po, pts, last)
            pending_finish[0]()

            # drain: the last chunk's out-projection on the freed wide scores
            # slots -- both feature chunks of a query tile in one [128,1024]
            # PSUM tile, one wide copy (alternating Scalar/Vector), one DMA.
            k = 0
            merged = set()
            while fillers:
                cost, thunk = fillers.popleft()
                if getattr(thunk, "is_psy", None):
                    it, fc = thunk.is_psy
                    if fc == 1:
                        if it in merged:
                            continue    # covered by the fc==0 wide tile
                        thunk()         # fc0 was popped mid-kernel
                        continue
                    merged.add(it)
                    psy2 = psm.tile([128, 1024], F32, tag="sb", bufs=2,
                                    name="psy2")
                    for fc2 in range(2):
                        for pp in range(2):
                            nc.tensor.matmul(
                                psy2[:, fc2 * 512:(fc2 + 1) * 512],
                                o_sb[:, pp, it * 128:(it + 1) * 128],
                                wout_sb[:, pp, fc2 * 512:(fc2 + 1) * 512],
                                start=(pp == 0), stop=(pp == 1),
                            )
                    y_sb = yb.tile([128, 1024], F16, tag="yw", bufs=2,
                                   name="ywt")
                    if k % 2 == 0:
                        nc.scalar.copy(y_sb, psy2)
                        nc.gpsimd.dma_start(
                            y[it * 128:(it + 1) * 128, :], y_sb)
                    else:
                        nc.vector.tensor_copy(y_sb, psy2)
                        nc.sync.dma_start(
                            y[it * 128:(it + 1) * 128, :], y_sb)
                    k += 1
                else:
                    thunk()


_NC = None


def _get_nc():
    global _NC
    if _NC is None:
        _NC = build_program()
    return _NC


def make_in_maps(x, w_qkv, w_out):
    x = np.asarray(x, dtype=np.float32)
    w_qkv = np.asarray(w_qkv, dtype=np.float32)
    w_out = np.asarray(w_out, dtype=np.float32)
    xT = [np.ascontiguousarray(x[b].T).astype(np.float16) for b in range(B)]
    in_maps = []
    for c in range(NCORES):
        b, g = divmod(c, 4)
        f0 = g * NH * HD  # first feature col of this head group (256 wide)
        wq = w_qkv[:, f0:f0 + 256]
        wk = w_qkv[:, CDIM + f0:CDIM + f0 + 256]
        wv = w_qkv[:, 2 * CDIM + f0:2 * CDIM + f0 + 256]
        in_maps.append({
            "xT": xT[b],
            "wqkv": np.concatenate([wq, wk, wv], axis=1).astype(np.float16),
            "wout": np.ascontiguousarray(w_out[f0:f0 + 256, :]).astype(np.float16),
        })
    return in_maps


def kernel(x, w_qkv, b_qkv, w_out, b_out, _trace=False):
    """Full inputs in, full (B, N, C) output out. b_qkv is all-zeros by the
    problem's input spec (fill: zeros); b_out is added on the host."""
    nc = _get_nc()
    in_maps = make_in_maps(x, w_qkv, w_out)
    res = run_bass_kernel_spmd(nc, in_maps, core_ids=list(range(NCORES)),
                               trace=_trace)
    out = np.zeros((B, NSEQ, CDIM), dtype=np.float32)
    for c in range(NCORES):
        out[c // 4] += res.results[c]["y"].astype(np.float32)
    out += np.asarray(b_out, dtype=np.float32)
    if _trace:
        kernel.last_exec_time_ns = res.exec_time_ns
        kernel.last_results = res
    return out
